# revision 40
# baseline (speedup 1.0000x reference)
"""Multi-head attention (12 heads, d_model=768, b=2, p=2048) on 8 TRN2 cores.

Sharding: core = i_b * 4 + i_g  (i_b: batch half, i_g: group of 3 heads).
Each core: qkv projection for its 3 heads, full softmax attention (faithful
to the source bug: logits *= sqrt(64) = 8, folded into Wq/bq on host), and a
partial out-projection [2048, 768]. Host sums the 4 group partials per batch
and adds bout.

Device pipeline per core (f32r matmuls unless noted):
  P1  qkvT chunks = Wqkv_slice.T @ x.T -> q8T/kT rows 0-63 of q_aug/k_aug
      [65, 2048] (+bias on eviction), vT [64, 2048] fp16
  P2  stats: logits8 [q-tile, k] -> DVE rowmax -> -B -> PE transpose -> DMA
      into q_aug row 64.  (B only needs +-85 abs accuracy; the row-shared
      shift cancels exactly in softmax.)
  P3  per (qhalf, kchunk): logitsT = k_aug.T @ q_aug (ones x -B row makes it
      pre-shifted) -> ACT exp -> fp16 -> attn@v accumulate; v_aug carries a
      ones column so psum row 64 accumulates the softmax denominator S.
  P4  recip(S) via PE transposes, DRAM-broadcast to [64, 2048], normalize
      outT in place, out_proj accumulate 3 heads, DMA out.
"""

import os

# The bass kernel executes through jax's axon PJRT backend; a JAX_PLATFORMS=cpu
# pin (common for running the pure-jax reference) would hide the NeuronCores.
if os.environ.get("JAX_PLATFORMS") == "cpu":
    os.environ["JAX_PLATFORMS"] = ""

import numpy as np

import concourse.bass as bass
import concourse.mybir as mybir
import concourse.tile as tile
from concourse import bacc
from concourse.bass import ds, ts
from concourse.bass_utils import run_bass_kernel_spmd
from concourse.masks import make_identity

P = 2048
D = 768
E = 576  # 3 heads x (q,k,v) x 64
KC = 16  # 128-wide chunks of P
F32 = mybir.dt.float32
F32R = mybir.dt.float32r
F16 = mybir.dt.float16

_CACHE = {}


def _trace(nc, tc, xT_d, w_d, bias_d, wout_d, ones_d, y_d):
    AL = mybir.AluOpType

    with tc.tile_pool(name="singles", bufs=1) as singles, tc.tile_pool(
        name="qk", bufs=1
    ) as qk:
        id128 = singles.tile([128, 128], F32)
        make_identity(nc, id128)
        id64f = singles.tile([64, 64], F32)
        make_identity(nc, id64f)
        id64r = singles.tile([64, 64], F32R)
        nc.vector.tensor_copy(id64r, id64f)
        id16f = singles.tile([16, 16], F32)
        make_identity(nc, id16f)
        id16r = singles.tile([16, 16], F32R)
        nc.vector.tensor_copy(id16r, id16f)
        bias_sb = singles.tile([128, 5], F32)
        nc.sync.dma_start(out=bias_sb, in_=bias_d)
        wout3 = [
            singles.tile([64, D], F32R, name=f"wout{h}") for h in range(3)
        ]

        q_aug = [qk.tile([65, P], F32R, name=f"q_aug{h}") for h in range(3)]
        k_aug = [qk.tile([65, P], F32R, name=f"k_aug{h}") for h in range(3)]
        vT = [qk.tile([64, P], F32R, name=f"vT{h}") for h in range(3)]
        outU = [qk.tile([65, P], F32R, name=f"outU{h}") for h in range(3)]
        recipB_t = qk.tile([64, P], F32, name="recipB")
        recipB = [recipB_t] * 3
        ones_col = bass.AP(
            tensor=ones_d.tensor, offset=ones_d.offset, ap=[[0, 128], [1, 1]]
        )

        with (
            tc.tile_pool(name="stps", bufs=2, space="PSUM") as stps,
            tc.tile_pool(name="lgps", bufs=2, space="PSUM") as lgps,
            tc.tile_pool(name="avps", bufs=1, space="PSUM") as avps,
            tc.tile_pool(name="asb", bufs=3) as asb,
            tc.tile_pool(name="ssb", bufs=4) as ssb,
            tc.tile_pool(name="op_sb", bufs=2) as op_sb,
            tc.tile_pool(name="drp", bufs=1, space="DRAM") as drp,
        ):
            # stats helpers ------------------------------------------------
            # b_neg[:, t] = -B[q-tile t].  DVE tiles: B = rowmax.  ACT tiles:
            # B = 16*ln2*floor(log2(sum exp(x/16))) in [max-11.1, max+122]
            # (f32r expT keeps tiny top-weights in normal fp32 range).
            ACT_TILES = {
                0: {0, 2, 4, 6, 8, 10, 12, 14},
                1: {5, 10, 15},
                2: {5, 10, 15},
            }
            b_negs = {}
            stat_parts = {}

            def emit_stat_quarter(h, t, quar):
                if t == 0 and quar == 0:
                    b_negs[h] = ssb.tile(
                        [128, KC], F32, name=f"b_neg{h}", tag=f"b_neg{h}"
                    )
                use_act = t in ACT_TILES[h]
                sps = stps.tile([128, 512], F32, name="stat", tag="stat")
                nc.tensor.matmul(
                    sps,
                    q_aug[h][0:64, ts(t, 128)],
                    k_aug[h][0:64, ts(quar, 512)],
                    start=True,
                    stop=True,
                )
                if use_act:
                    scr = asb.tile([128, 512], F32, name="scr", tag="scr")
                    sq = ssb.tile([128, 1], F32, name="sq", tag=f"sq{quar}")
                    nc.scalar.activation(
                        out=scr,
                        in_=sps,
                        func=mybir.ActivationFunctionType.Exp,
                        scale=0.0625,
                        accum_out=sq,
                    )
                else:
                    sq = ssb.tile([128, 1], F32, name="mx", tag=f"mx{quar}")
                    nc.vector.reduce_max(
                        sq, sps, axis=mybir.AxisListType.X, negate=True
                    )
                stat_parts.setdefault((h, t), []).append(sq)

            def emit_stat_finalize(h, t):
                parts = stat_parts.pop((h, t))
                use_act = t in ACT_TILES[h]
                op = AL.add if use_act else AL.min
                m01 = ssb.tile([128, 1], F32, name="m01", tag="m01")
                m23 = ssb.tile([128, 1], F32, name="m23", tag="m23")
                nc.vector.tensor_tensor(m01, parts[0], parts[1], op=op)
                nc.vector.tensor_tensor(m23, parts[2], parts[3], op=op)
                if use_act:
                    stot = ssb.tile([128, 1], F32, name="stot", tag="stot")
                    nc.vector.tensor_tensor(stot, m01, m23, op=AL.add)
                    sh = ssb.tile([128, 1], mybir.dt.uint32, name="sh", tag="sh")
                    nc.vector.tensor_scalar(
                        out=sh,
                        in0=stot.bitcast(mybir.dt.uint32),
                        scalar1=23,
                        scalar2=None,
                        op0=AL.logical_shift_right,
                    )
                    nc.vector.tensor_scalar(
                        out=b_negs[h][:, t : t + 1],
                        in0=sh,
                        scalar1=127.0,
                        scalar2=-11.0904,
                        op0=AL.subtract,
                        op1=AL.mult,
                    )
                else:
                    nc.vector.tensor_tensor(
                        b_negs[h][:, t : t + 1], m01, m23, op=AL.min
                    )

            def emit_stat_tile(h, t):
                for quar in range(4):
                    emit_stat_quarter(h, t, quar)
                emit_stat_finalize(h, t)

            def emit_b_seg(h, seg):
                """Transpose b_neg[:, 4seg:4seg+4] into q_aug row 64 segment."""
                bt_ps = stps.tile([4, 128], F32, name="bt_ps", tag="stat")
                nc.tensor.transpose(bt_ps, b_negs[h][:, ds(seg * 4, 4)], id128)
                bt_sb = ssb.tile([4, 128], F32R, name="bt_sb", tag=f"bt_sb{seg}")
                nc.vector.tensor_copy(bt_sb, bt_ps)
                nc.sync.dma_start(
                    out=q_aug[h][64:65, ds(seg * 512, 512)], in_=bt_sb
                )

            v_augs = {}
            av_tiles = {}

            def emit_p3_chunk(h, qh, kc):
                if kc == 0:
                    av_tiles[(h, qh)] = avps.tile(
                        [65, 1024], F32, name="av", tag="av"
                    )
                av = av_tiles[(h, qh)]
                lg = lgps.tile([128, 1024], F32, name="lg", tag="lg")
                for n in range(2):
                    nc.tensor.matmul(
                        lg[:, ts(n, 512)],
                        k_aug[h][:, ts(kc, 128)],
                        q_aug[h][:, ds(qh * 1024 + n * 512, 512)],
                        start=True,
                        stop=True,
                    )
                ex = asb.tile([128, 1024], F32R, name="ex", tag="ex")
                nc.scalar.activation(
                    out=ex, in_=lg, func=mybir.ActivationFunctionType.Exp
                )
                for n in range(2):
                    nc.tensor.matmul(
                        av[:, ts(n, 512)],
                        v_augs[(h, kc)],
                        ex[:, ts(n, 512)],
                        start=(kc == 0),
                        stop=(kc == KC - 1),
                    )
                if kc == KC - 1:
                    nc.vector.tensor_copy(
                        outU[h][:, ds(qh * 1024, 1024)], av_tiles.pop((h, qh))
                    )

            def emit_v_aug(h, kc):
                vt_ps = stps.tile([128, 64], F32R, name="vt_ps", tag="stat")
                nc.tensor.transpose(vt_ps, vT[h][:, ts(kc, 128)], id64r)
                va = asb.tile(
                    [128, 65], F32R, name=f"va{h}_{kc}", tag=f"va{h}_{kc}", bufs=1
                )
                nc.vector.tensor_copy(va[:, 0:64], vt_ps)
                nc.sync.dma_start(out=va[:, 64:65], in_=ones_col)
                v_augs[(h, kc)] = va

            # ------------- P1: qkvT (stats(0) + v_aug interleaved) --------
            # e-rows of qkvT: 64-row subtensor s (0..8) -> head s//3,
            # kind s%3 (0:q8, 1:k, 2:v).  Processed in two p-halves (nh) so
            # only half of xT is resident; stats(0) runs during nh=1.
            with tc.tile_pool(name="p1sb", bufs=1) as p1sb:
                w_sb = p1sb.tile([128, 6, E], F32R)
                nc.sync.dma_start(out=w_sb[:, 0, :], in_=w_d[0:128, :])
                def sq_fin(ts_, quars, fin):
                    out = []
                    for t in ts_:
                        out += [("sq", 0, t, q) for q in quars]
                        if fin:
                            out.append(("fin", 0, t))
                    return out

                inter = {
                    (0, 0): sq_fin([0, 1, 2], (0, 1), False),
                    (0, 1): sq_fin([3, 4, 5], (0, 1), False) + [("va", 0, kc) for kc in range(0, 4)],
                    (0, 2): sq_fin([6, 7], (0, 1), False) + [("va", 0, kc) for kc in range(4, 8)] + [("va", 1, kc) for kc in range(0, 2)],
                    (0, 3): [("va", 1, kc) for kc in range(2, 6)],
                    (0, 4): [("va", 2, kc) for kc in range(0, 4)],
                    (1, 0): sq_fin([0, 1, 2, 3], (2, 3), True) + [("bseg", 0, 0)],
                    (1, 1): sq_fin([4, 5, 6, 7], (2, 3), True) + [("bseg", 0, 1)] + [("va", 0, kc) for kc in range(8, 16)],
                    (1, 2): sq_fin([8, 9, 10], (0, 1, 2, 3), True) + [("va", 1, kc) for kc in range(6, 16)],
                    (1, 3): sq_fin([11, 12, 13], (0, 1, 2, 3), True) + [("bseg", 0, 2)] + [("va", 2, kc) for kc in range(4, 8)] + [("p3", 0, kc) for kc in range(0, 6)],
                    (1, 4): sq_fin([14, 15], (0, 1, 2, 3), True) + [("bseg", 0, 3)] + [("va", 2, kc) for kc in range(8, 16)] + [("p3", 0, kc) for kc in range(6, 12)],
                }
                for nh in range(2):
                    xT_sb = p1sb.tile([128, 6, 1024], F32R, name="xT_sb", tag="xT_sb")
                    for c in range(6):
                        nc.sync.dma_start(
                            out=xT_sb[:, c, :],
                            in_=xT_d[ds(c * 128, 128), ds(nh * 1024, 1024)],
                        )
                    if nh == 0:
                        for c in range(1, 6):
                            nc.sync.dma_start(
                                out=w_sb[:, c, :], in_=w_d[ds(c * 128, 128), :]
                            )
                        for hh in range(3):
                            nc.sync.dma_start(
                                out=k_aug[hh][64:65, :], in_=ones_d
                            )
                        for hh in range(3):
                            nc.sync.dma_start(
                                out=wout3[hh], in_=wout_d[ds(64 * hh, 64), :]
                            )
                    for j in range(5):
                        rows = 128 if j < 4 else 64
                        ps = lgps.tile([128, 1024], F32, name="qkvps", tag="lg")
                        for c in range(6):
                            for n in range(2):
                                nc.tensor.matmul(
                                    ps[0:rows, ts(n, 512)],
                                    w_sb[:, c, ds(j * 128, rows)],
                                    xT_sb[:, c, ts(n, 512)],
                                    start=(c == 0),
                                    stop=(c == 5),
                                )
                        for half in range(rows // 64):
                            sub = 2 * j + half
                            hh, kind = divmod(sub, 3)
                            dst = (q_aug[hh], k_aug[hh], vT[hh])[kind]
                            if kind != 2:
                                dst = dst[0:64, :]
                            nc.vector.tensor_scalar(
                                out=dst[:, ds(nh * 1024, 1024)],
                                in0=ps[ds(half * 64, 64), :],
                                scalar1=bias_sb[ds(half * 64, 64), j : j + 1],
                                scalar2=None,
                                op0=AL.add,
                            )
                        for item in inter.get((nh, j), []):
                            if item[0] == "sq":
                                emit_stat_quarter(item[1], item[2], item[3])
                            elif item[0] == "fin":
                                emit_stat_finalize(item[1], item[2])
                            elif item[0] == "bseg":
                                emit_b_seg(item[1], item[2])
                            elif item[0] == "p3":
                                emit_p3_chunk(0, 0, item[2])
                            else:
                                emit_v_aug(item[1], item[2])

            # ------------- attention ------------------------------------
            def emit_p4a(h, qh=None):
                # ---- P4a: normalize outT[:, half] by 1/S ----
                halves = (0, 1) if qh is None else (qh,)
                for hf in halves:
                    s_sb = ssb.tile([16, 64], F32R, name="s_sb", tag="s_sb")
                    nc.sync.dma_start(
                        out=s_sb, in_=outU[h][64:65, ds(hf * 1024, 1024)]
                    )
                    s_col = stps.tile([64, 16], F32R, name="s_col", tag="stat")
                    nc.tensor.transpose(s_col, s_sb, id16r)
                    rc = ssb.tile([64, 16], F32, name="rc", tag="rc")
                    nc.vector.reciprocal(rc, s_col.bitcast(F32))
                    rt_ps = stps.tile([16, 64], F32, name="rt_ps", tag="stat")
                    nc.tensor.transpose(rt_ps, rc, id64f)
                    rt_sb = ssb.tile([16, 64], F32, name="rt_sb", tag="rt_sb")
                    nc.vector.tensor_copy(rt_sb, rt_ps)
                    r_dram = drp.tile([1024], F32, name="r_dram", tag=f"r_dram{h}_{hf}")
                    nc.sync.dma_start(out=r_dram, in_=rt_sb)
                    nc.gpsimd.dma_start(
                        out=recipB[h][:, ds(hf * 1024, 1024)],
                        in_=bass.AP(
                            tensor=r_dram.tensor,
                            offset=r_dram.offset,
                            ap=[[0, 64], [1, 1024]],
                        ),
                    )
                    nc.vector.tensor_tensor(
                        out=outU[h][0:64, ds(hf * 1024, 1024)],
                        in0=outU[h][0:64, ds(hf * 1024, 1024)],
                        in1=recipB[h][:, ds(hf * 1024, 1024)],
                        op=AL.mult,
                    )

            def emit_out_proj(t, opsb_pool):
                """out_proj tile t via two [128, 384] psum halves in stat slots."""
                yo = opsb_pool.tile([128, D], F32, name="yo", tag="yo")
                for pi in range(2):
                    po = stps.tile([128, 384], F32, name="po", tag="stat")
                    for h in range(3):
                        nc.tensor.matmul(
                            po,
                            outU[h][0:64, ts(t, 128)],
                            wout3[h][:, ds(pi * 384, 384)],
                            start=(h == 0),
                            stop=(h == 2),
                        )
                    nc.vector.tensor_copy(yo[:, ds(pi * 384, 384)], po)
                nc.sync.dma_start(out=y_d[ts(t, 128), :], in_=yo)

            for h in range(3):
                sq_sched = []
                if h < 2:
                    for t in range(KC):
                        for quar in range(4):
                            sq_sched.append((h + 1, t, quar, quar == 3))
                ci = 0
                for qh in range(2):
                    for kc in range(KC):
                        if h == 0 and qh == 0 and kc < 12:
                            pass  # emitted inside P1 interleave
                        else:
                            emit_p3_chunk(h, qh, kc)
                        if h > 0 and qh == 0 and kc == 6:
                            emit_p4a(h - 1)
                        if h == 2 and qh == 1 and 3 <= kc < 11:
                            emit_out_proj(kc - 3, op_sb)
                        for sqi in (ci * 2, ci * 2 + 1):
                            if sq_sched and sqi < len(sq_sched):
                                hh, t, quar, fin = sq_sched[sqi]
                                emit_stat_quarter(hh, t, quar)
                                if fin:
                                    emit_stat_finalize(hh, t)
                                    if t % 4 == 3:
                                        emit_b_seg(hh, t // 4)
                        ci += 1
                    if h == 2 and qh == 0:
                        emit_p4a(2, 0)

            emit_p4a(2, 1)
            for t in range(8, KC):
                emit_out_proj(t, op_sb)


def _build():
    nc = bacc.Bacc("TRN2", target_bir_lowering=False, debug=False, num_devices=8)
    xT_d = nc.dram_tensor("xT", [D, P], F32R, kind="ExternalInput").ap()
    w_d = nc.dram_tensor("w", [D, E], F32R, kind="ExternalInput").ap()
    bias_d = nc.dram_tensor("bias", [128, 5], F32, kind="ExternalInput").ap()
    wout_d = nc.dram_tensor("wout", [192, D], F32R, kind="ExternalInput").ap()
    ones_d = nc.dram_tensor("ones", [1, P], F32R, kind="ExternalInput").ap()
    y_d = nc.dram_tensor("y", [P, D], F32, kind="ExternalOutput").ap()

    with tile.TileContext(nc) as tc:
        _trace(nc, tc, xT_d, w_d, bias_d, wout_d, ones_d, y_d)
    nc.compile()
    return nc


def make_in_maps(x, Wqkv, bqkv, Wout):
    xT = np.ascontiguousarray(x.transpose(0, 2, 1))  # [2, 768, 2048]
    in_maps = []
    for core in range(8):
        i_b, i_g = divmod(core, 4)
        cols = slice(E * i_g, E * (i_g + 1))
        w = Wqkv[:, cols].copy()
        bv = bqkv[cols].copy()
        for j in range(3):  # fold logits *= sqrt(64) into q columns
            w[:, j * 192 : j * 192 + 64] *= 8.0
            bv[j * 192 : j * 192 + 64] *= 8.0
        bias_packed = np.zeros((128, 5), dtype=np.float32)
        for j in range(5):
            rows = 128 if j < 4 else 64
            bias_packed[0:rows, j] = bv[j * 128 : j * 128 + rows]
        in_maps.append(
            {
                "xT": np.ascontiguousarray(xT[i_b]),
                "w": np.ascontiguousarray(w),
                "bias": bias_packed,
                "wout": np.ascontiguousarray(Wout[192 * i_g : 192 * (i_g + 1), :]),
                "ones": np.ones((1, P), dtype=np.float32),
            }
        )
    return in_maps


def kernel(x, Wqkv, bqkv, Wout, bout, _run_kwargs=None):
    x = np.asarray(x, dtype=np.float32)
    Wqkv = np.asarray(Wqkv, dtype=np.float32)
    bqkv = np.asarray(bqkv, dtype=np.float32)
    Wout = np.asarray(Wout, dtype=np.float32)
    bout = np.asarray(bout, dtype=np.float32)

    if "nc" not in _CACHE:
        _CACHE["nc"] = _build()
    nc = _CACHE["nc"]

    in_maps = make_in_maps(x, Wqkv, bqkv, Wout)
    res = run_bass_kernel_spmd(
        nc, in_maps, core_ids=list(range(8)), **(_run_kwargs or {})
    )
    out = np.zeros((2, P, D), dtype=np.float32)
    for core in range(8):
        out[core // 4] += res.results[core]["y"]
    out += bout
    if _run_kwargs:
        _CACHE["last_res"] = res
    return out
